# revision 1
# baseline (speedup 1.0000x reference)
"""Causal single-head attention (B=64, T=512, D=768, H=96) on 8 TRN2 NeuronCores.

Data-parallel: core c computes x[8c:8c+8] with replicated weights; no
collectives. Cost-model (TimelineSim) time: ~79.5 us/core; rel err ~3e-4
(f32r = tf32-like 12-bit-mantissa input rounding, full fp32 accumulate).

Per-batch pipeline on one core:
  x --PE-transpose (f32r, 24x 128x128)--> xT[d, t]
  qT/kT/vT = W.T @ xT        (f32r matmuls: 1 cyc/row at N>=256 vs 4 for fp32)
  scoresT_j[tk, tq>=128j] = kT_j.T @ qT     (causal truncation in the free dim)
  expT = ACT Exp(scale*scoresT) -> f32r; GPSIMD multiplies the diagonal block
  by an upper-triangular 0/1 mask (no max-subtraction: |scores/sqrt(96)| = O(6))
  vT --PE-transpose--> v1 = [v | ones]      (ones column = softmax denominator)
  outT[0:97, tq] += v1_j.T @ expT_j         (row 96 accumulates sum_tk exp)
  outT --PE-transpose--> psum[tq, 0:97]; out = psum[:, :96] * recip(psum[:, 96])
The finish phase (transpose-back + normalize + store) of batch b is deferred
into batch b+1 to fill PE waits. Engine split: PE matmuls/transposes, DVE
xT copies + normalize, ACT qkv/outT copies + exp, GPSIMD masks/memsets.
"""

import numpy as np

import concourse.bass as bass
import concourse.mybir as mybir
import concourse.tile as tile
from concourse.masks import make_identity, make_upper_triangular

B, T, D, H = 64, 512, 768, 96
N_CORES = 8
BP = B // N_CORES  # batches per core
P = 128
DC = D // P  # 6 contraction chunks
TC = T // P  # 4 sequence chunks
SCALE = 1.0 / float(np.sqrt(H))
F32 = mybir.dt.float32
F32R = mybir.dt.float32r
USE_F32R = True

XSPLIT = 2  # x DMAs per batch (tuned: 1 and 3+ both slower)


def _r(ap):
    return ap.bitcast(F32R) if USE_F32R else ap


def _split_excess_waits(nc: bass.Bass, limit: int = 1) -> None:
    """This walrus build rejects instructions with more than one sync-wait
    command ("Too many sync wait commands" in setupSyncWait). Move excess
    waits onto preceding single-wait NoOps on the same engine — the engine
    processes instructions in order, so blocking semantics are preserved."""
    k = 0
    for f in nc.m.functions:
        for blk in f.blocks:
            out = []
            for inst in blk.instructions:
                si = inst.sync_info
                if si is not None and len(si.on_wait) > limit:
                    # engine sems complete early, DMA-queue sems last: order
                    # the serial single-wait NoOps so dispatch overhead hides
                    # before the longest wait instead of trailing it
                    waits = sorted(
                        si.on_wait,
                        key=lambda w: ((w.ant_name or "").startswith("DMA"), ),
                    )
                    for w in waits[:-limit]:
                        nop = mybir.InstNoOp(name=f"WSPLIT-{k}", engine=inst.engine)
                        k += 1
                        nop.sync_info = mybir.SyncInfo(on_wait=[w], on_update=[])
                        out.append(nop)
                    inst.sync_info = mybir.SyncInfo(
                        on_wait=waits[-limit:], on_update=list(si.on_update)
                    )
                out.append(inst)
            blk.instructions = out


def build_bass(repeat: int = 1) -> bass.Bass:
    nc = bass.Bass(name="attn_dp")
    x = nc.dram_tensor("x", (BP, T, D), F32, kind="ExternalInput")
    wq = nc.dram_tensor("Wq", (D, H), F32, kind="ExternalInput")
    wk = nc.dram_tensor("Wk", (D, H), F32, kind="ExternalInput")
    wv = nc.dram_tensor("Wv", (D, H), F32, kind="ExternalInput")
    out = nc.dram_tensor("out", (BP, T, H), F32, kind="ExternalOutput")

    with tile.TileContext(nc) as tc:
        with (
            tc.tile_pool(name="consts", bufs=1) as consts,
            tc.tile_pool(name="xin", bufs=3) as xin,
            tc.tile_pool(name="xtp", bufs=2) as xtp,
            tc.tile_pool(name="qkv", bufs=2) as qkv,
            tc.tile_pool(name="expp", bufs=2) as expp,
            tc.tile_pool(name="v1p", bufs=8) as v1p,
            tc.tile_pool(name="otp", bufs=2) as otp,
            tc.tile_pool(name="outp", bufs=8) as outp,
            tc.tile_pool(name="ps_xt", bufs=2, space="PSUM") as ps_xt,
            tc.tile_pool(name="ps_proj", bufs=2, space="PSUM") as ps_proj,
            tc.tile_pool(name="ps_sc", bufs=3, space="PSUM") as ps_sc,
        ):
            ident = consts.tile([P, P], F32)
            make_identity(nc, ident)
            # f32r-rounded identity for f32r-mode transposes (exact: 0/1)
            ident_r = consts.tile([P, P], F32, tag="ident_r")
            nc.vector.tensor_copy(_r(ident_r), ident)
            # keep-mask for the diagonal block of scoresT[tk, tq]: 1 where tk<=tq
            tri = consts.tile([P, P], F32)
            make_upper_triangular(nc, tri, val=1.0, diag=True)
            # f32r-typed ones column (memset can't write f32r; copy rounds)
            ones_f = consts.tile([P, 1], F32, tag="ones_f")
            nc.gpsimd.memset(ones_f, 1.0)

            w_sb = []
            for name, w in (("wq", wq), ("wk", wk), ("wv", wv)):
                t = consts.tile([P, DC, H], F32, tag=name)
                # issue on the ACT HWDGE queue so x[0] isn't queued behind
                nc.scalar.dma_start(
                    out=_r(t), in_=_r(w.rearrange("(o p) h -> p o h", p=P))
                )
                w_sb.append(t)

            pending = None  # (ot_sb, b) finish-work deferred one batch
            _rep = repeat

            def emit_finish(ot_sb, b):
                o_all = outp.tile([P, TC, H], F32)
                for i in range(TC):
                    tr_ps = ps_sc.tile([P, H + 1], F32, tag="sc_ps")
                    nc.tensor.transpose(
                        tr_ps,
                        ot_sb[:, i * P : (i + 1) * P],
                        ident[: H + 1, : H + 1],
                    )
                    rec = outp.tile([P, 1], F32, tag="rec")
                    nc.vector.reciprocal(rec, tr_ps[:, H : H + 1])
                    nc.vector.tensor_scalar_mul(o_all[:, i, :], tr_ps[:, :H], rec)
                nc.sync.dma_start(
                    out=out[b].rearrange("(i p) h -> p i h", p=P), in_=o_all
                )

            for b_all in range(BP * _rep):
                b = b_all % BP
                # ---- load x[b] as [p, tc, d] ----
                x_sb = xin.tile([P, TC, D], F32)
                xr = x[b].rearrange("(i p) d -> p i d", p=P)
                step = D // XSPLIT
                for s in range(XSPLIT):
                    nc.sync.dma_start(
                        out=_r(x_sb[:, :, s * step : (s + 1) * step]),
                        in_=_r(xr[:, :, s * step : (s + 1) * step]),
                    )

                xt_sb = xtp.tile([P, DC, T], F32)

                def emit_T(d):
                    # transpose x d-chunk -> xT[:, d, :]
                    xt_ps = ps_xt.tile([P, T], F32, tag="xt_ps")
                    for i in range(TC):
                        nc.tensor.transpose(
                            _r(xt_ps[:, i * P : (i + 1) * P]),
                            _r(x_sb[:, i, d * P : (d + 1) * P]),
                            _r(ident_r),
                        )
                    # rounds to f32r during the PSUM->SBUF copy (DVE: tuned)
                    nc.vector.tensor_copy(_r(xt_sb[:, d, :]), xt_ps)

                def emit_proj(pp, pi, d):
                    nc.tensor.matmul(
                        pp,
                        lhsT=_r(w_sb[pi][:, d, :]),
                        rhs=_r(xt_sb[:, d, :]),
                        start=(d == 0),
                        stop=(d == DC - 1),
                    )

                last = b_all >= BP * _rep - 2

                def qkv_copy(tag, pp, dve=False):
                    t = qkv.tile([H, T], F32, tag=tag)
                    if dve:
                        nc.vector.tensor_copy(_r(t), pp)
                    else:
                        nc.scalar.copy(out=_r(t), in_=pp)
                    return t

                for d in range(DC):
                    emit_T(d)
                qp = ps_proj.tile([H, T], F32, tag="proj_ps")
                for d in range(DC):
                    emit_proj(qp, 0, d)
                qT_sb = qkv_copy("pT0", qp)

                kp = ps_proj.tile([H, T], F32, tag="proj_ps")
                for d in range(DC):
                    emit_proj(kp, 1, d)
                kT_sb = qkv_copy("pT1", kp, dve=last)

                vp = ps_proj.tile([H, T], F32, tag="proj_ps")
                for d in range(DC):
                    emit_proj(vp, 2, d)
                vT_sb = qkv_copy("pT2", vp, dve=last)

                # previous batch's finish work fills the ACT-copy wait here
                if pending is not None:
                    emit_finish(*pending)
                    pending = None

                # ---- scoresT per tk-chunk, exp, diagonal causal mask ----
                # j=3 is padded from N=128 to N=256: f32r matmuls below 256
                # moving rows run at 4 cyc/row, so the extra fully-masked 128
                # columns make the matmul 2x faster (256 vs 512 cycles)
                eT = [None] * TC
                for j in range(TC):
                    tq0 = (j if j < TC - 1 else j - 1) * P
                    nj = T - tq0
                    sc_ps = ps_sc.tile([P, T], F32, tag="sc_ps")
                    nc.tensor.matmul(
                        sc_ps[:, :nj],
                        lhsT=_r(kT_sb[:, j * P : (j + 1) * P]),
                        rhs=_r(qT_sb[:, tq0:]),
                        start=True,
                        stop=True,
                    )
                    et = expp.tile([P, nj], F32, tag=f"exp{j}")
                    nc.scalar.activation(
                        out=_r(et),
                        in_=sc_ps[:, :nj],
                        func=mybir.ActivationFunctionType.Exp,
                        scale=SCALE,
                    )
                    pad = j * P - tq0  # leading fully-masked columns
                    if pad:
                        nc.gpsimd.memset(et[:, :pad], 0.0)
                    nc.gpsimd.tensor_tensor(
                        out=_r(et[:, pad : pad + P]), in0=et[:, pad : pad + P],
                        in1=tri, op=mybir.AluOpType.mult,
                    )
                    eT[j] = et

                # ---- v natural [tk, h] with appended f32r ones column ----
                v1 = []
                for j in range(TC):
                    vt_ps = ps_sc.tile([P, H], F32, tag="sc_ps")
                    nc.tensor.transpose(
                        _r(vt_ps),
                        _r(vT_sb[:, j * P : (j + 1) * P]),
                        _r(ident_r[:H, :H]),
                    )
                    v1t = v1p.tile([P, H + 1], F32, tag="v1")
                    nc.gpsimd.tensor_copy(out=_r(v1t[:, H : H + 1]), in_=ones_f)
                    nc.vector.tensor_copy(_r(v1t[:, :H]), vt_ps)
                    v1.append(v1t)

                # ---- outT[0:97, tq] accumulated over tk chunks ----
                o_ps = ps_sc.tile([H + 1, T], F32, tag="sc_ps")
                for jj, j in enumerate(range(TC)):
                    tq0 = (j if j < TC - 1 else j - 1) * P
                    nc.tensor.matmul(
                        o_ps[:, tq0:],
                        lhsT=_r(v1[j]),
                        rhs=_r(eT[j]),
                        start=(jj == 0),
                        stop=(jj == TC - 1),
                    )
                ot_sb = otp.tile([H + 1, T], F32)
                nc.scalar.copy(out=ot_sb, in_=o_ps)
                pending = (ot_sb, b)

            if pending is not None:
                emit_finish(*pending)

    _split_excess_waits(nc)
    return nc


def kernel(x: np.ndarray, Wq: np.ndarray, Wk: np.ndarray, Wv: np.ndarray) -> np.ndarray:
    from concourse.bass_utils import run_bass_kernel_spmd

    x = np.ascontiguousarray(np.asarray(x, dtype=np.float32))
    Wq = np.ascontiguousarray(np.asarray(Wq, dtype=np.float32))
    Wk = np.ascontiguousarray(np.asarray(Wk, dtype=np.float32))
    Wv = np.ascontiguousarray(np.asarray(Wv, dtype=np.float32))

    in_maps = [
        {"x": x[c * BP : (c + 1) * BP], "Wq": Wq, "Wk": Wk, "Wv": Wv}
        for c in range(N_CORES)
    ]
    last_exc = None
    for attempt in range(3):
        try:
            nc = build_bass()
            res = run_bass_kernel_spmd(nc, in_maps, core_ids=list(range(N_CORES)))
            return np.concatenate([r["out"] for r in res.results], axis=0)
        except Exception as e:  # transient NRT/axon device errors
            last_exc = e
            import time as _time

            _time.sleep(2.0 * (attempt + 1))
    raise last_exc

